# revision 4
# baseline (speedup 1.0000x reference)
"""Bayesian linear layer on 8 Trainium2 NeuronCores.

Computes: weight = mu + softplus(rho) * eps  (elementwise, [O, I])
          bias   = b_mu + softplus(b_rho) * b_eps              ([O])
          y      = x @ weight.T + bias       ([N, I] @ [I, O] -> [N, O])

Shapes: x [8192, 4096], weight_* [16384, 4096], bias_* [16384].

Sharding: column-parallel over 8 cores — each core owns 2048 output
features (its slice of the weight/bias params), x is replicated. Each
core computes an independent [8192, 2048] output slice; the host
concatenates along the feature dim. No collectives needed.

Device kernel (SPMD, one Bass program, per-core data):
 - softplus computed as Ln(Exp(rho) + 1) on the scalar engine (the
   container's act tables lack a direct softplus entry).
 - weights materialized on-chip into resident bf16 SBUF tiles
   [128 i-part, 2048 o] x 32 k-tiles (128 KB/partition).
 - x streamed as transposed bf16 tiles [128 i-part, 32 kt, 128 n];
   host pre-transposes x (both matmul operands need the contraction
   dim on partitions; DMA transpose only supports 2-byte dtypes and
   strided f32 gathers are far off line-rate).
 - matmul: out[n, o] += xT_tile.T @ w_tile, PSUM [128 n, 2048 o]
   (4 banks), 32-step K accumulation, bias added during the PSUM->SBUF
   copy (one DVE pass), then DMA to DRAM.
"""

import numpy as np
import ml_dtypes

import concourse.bass as bass
import concourse.mybir as mybir
import concourse.tile as tile
from concourse.bass_utils import run_bass_kernel_spmd
from concourse.vector_clock import ScopedClock, VectorClock

N_CORES = 8
N_TOK = 8192
IN_F = 4096
OUT_F = 16384
O_PER = OUT_F // N_CORES  # 2048 out features per core

P = 128
KT = IN_F // P       # 32 k-tiles
MT = N_TOK // P      # 64 m-tiles
OC = 512             # o-chunk for weight materialization + matmul N
NOC = O_PER // OC    # 4 o-chunks

F32 = mybir.dt.float32
BF16 = mybir.dt.bfloat16
AF = mybir.ActivationFunctionType
ALU = mybir.AluOpType


def _patch_tile_drain():
    """The walrus build here caps sync-wait commands per CTRL_NO_STRUCT
    instruction; Tile's kernel-tail Drain overflows it. Spread the waits
    across nop carriers (one wait each) before the drain."""
    if getattr(tile.TileContext, "_drain_patched", False):
        return

    def _drain_and_barrier(self, tick_clock, wait_clock):
        nc = self.nc
        gc = tick_clock.global_clock
        n = len(gc)
        for i in range(n):
            t = gc[i]
            if t > 0:
                sub = [0] * n
                sub[i] = t
                carrier = nc.sync.nop(nofuse=True)
                wait_clock.add_sem_waits(
                    carrier.ins, ScopedClock({None: VectorClock(sub)})
                )
        nc.sync.drain()
        nc.all_engine_barrier()
        popped = nc._tile_sem_poison_stack.pop()
        assert popped is self._sem_poison
        nc.clear_and_free_semaphores(list(self.sems.allocated().values()))
        nc.all_engine_barrier()

    tile.TileContext._drain_and_barrier = _drain_and_barrier
    tile.TileContext._drain_patched = True


def _split_sync_waits(nc, max_waits=1):
    """This container's walrus build accepts at most ONE sync-wait command
    per instruction (a 2-wait TensorTensor fails codegen with 'Too many
    sync wait commands'). Tile emits up to 3. Spill the excess onto
    same-engine InstNoOp carriers inserted immediately before the
    overloaded instruction — same-engine program order preserves the
    wait-before-execute semantics."""
    n_spilled = 0
    for fn in nc.m.functions:
        for bb in fn.blocks:
            insts = list(bb.instructions)
            out = []
            changed = False
            for inst in insts:
                si = inst.sync_info
                if si is not None and si.on_wait and len(si.on_wait) > max_waits:
                    waits = list(si.on_wait)
                    spill, keep = waits[:-max_waits], waits[-max_waits:]
                    for w in spill:
                        nop = mybir.InstNoOp(
                            name=f"I-waitspill-{nc.next_id()}", ins=[], outs=[]
                        )
                        nop.engine = inst.engine
                        nop.sync_info = mybir.SyncInfo(on_wait=[w], on_update=[])
                        out.append(nop)
                        n_spilled += 1
                    inst.sync_info = mybir.SyncInfo(
                        on_wait=keep, on_update=list(si.on_update)
                    )
                    changed = True
                out.append(inst)
            if changed:
                bb.instructions = out
    return n_spilled


def _build():
    _patch_tile_drain()
    nc = bass.Bass()

    xT = nc.dram_tensor("xT", [IN_F, N_TOK], BF16, kind="ExternalInput")
    wmuT = nc.dram_tensor("wmuT", [IN_F, O_PER], F32, kind="ExternalInput")
    wrhoT = nc.dram_tensor("wrhoT", [IN_F, O_PER], F32, kind="ExternalInput")
    wepsT = nc.dram_tensor("wepsT", [IN_F, O_PER], F32, kind="ExternalInput")
    bmu = nc.dram_tensor("bmu", [1, O_PER], F32, kind="ExternalInput")
    brho = nc.dram_tensor("brho", [1, O_PER], F32, kind="ExternalInput")
    beps = nc.dram_tensor("beps", [1, O_PER], F32, kind="ExternalInput")
    y = nc.dram_tensor("y", [N_TOK, O_PER], F32, kind="ExternalOutput")

    xT_r = xT[:, :].rearrange("(kt p) n -> p kt n", p=P)

    with tile.TileContext(nc) as tc:
        with (
            tc.tile_pool(name="wpool", bufs=1) as wpool,
            tc.tile_pool(name="stage", bufs=2) as stage,
            tc.tile_pool(name="xpool", bufs=2) as xpool,
            tc.tile_pool(name="opool", bufs=2) as opool,
            tc.tile_pool(name="bpool", bufs=1) as bpool,
            tc.tile_pool(name="psum", bufs=2, space="PSUM") as psump,
        ):
            # resident bf16 weights: 32 x [128, 2048] = 128 KB/partition
            w_tiles = []
            for k in range(KT):
                wk = wpool.tile([P, O_PER], BF16, name=f"w_{k}", tag=f"w_{k}")
                w_tiles.append(wk)

            bias_bc = bpool.tile([P, O_PER], F32, name="bias_bc")

            def softplus_fma(dst, rho_src, eps_src, mu_src, exp_t, sp_t, prod_t):
                # dst = mu + softplus(rho) * eps, via Ln(Exp(rho) + 1)
                nc.scalar.activation(exp_t, rho_src, AF.Exp)
                nc.scalar.activation(sp_t, exp_t, AF.Ln, bias=1.0)
                nc.vector.tensor_mul(prod_t, sp_t, eps_src)
                nc.vector.tensor_add(dst, prod_t, mu_src)

            # ── bias: compute softplus fma on partition 0, then replicate
            # to all 128 partitions via a doubling SBUF->SBUF DMA ladder
            # (the InstPartitionBroadcast custom op fails codegen here).
            for oc in range(NOC):
                sl = bass.ts(oc, OC)
                rho_s = stage.tile([P, OC], F32, name="rho_s", tag="rho_s")
                eps_s = stage.tile([P, OC], F32, name="eps_s", tag="eps_s")
                mu_s = stage.tile([P, OC], F32, name="mu_s", tag="mu_s")
                exp_s = stage.tile([P, OC], F32, name="exp_s", tag="exp_s")
                sp_s = stage.tile([P, OC], F32, name="sp_s", tag="sp_s")
                nc.sync.dma_start(rho_s[0:1, :], brho[0:1, sl])
                nc.sync.dma_start(eps_s[0:1, :], beps[0:1, sl])
                nc.sync.dma_start(mu_s[0:1, :], bmu[0:1, sl])
                softplus_fma(
                    bias_bc[0:1, sl], rho_s[0:1, :], eps_s[0:1, :], mu_s[0:1, :],
                    exp_s[0:1, :], rho_s[0:1, :], exp_s[0:1, :],
                )
            k = 1
            while k < P:
                nc.sync.dma_start(bias_bc[k : 2 * k, :], bias_bc[0:k, :])
                k *= 2

            # ── weight materialization
            for k in range(KT):
                ksl = slice(k * P, (k + 1) * P)
                for oc in range(NOC):
                    sl = bass.ts(oc, OC)
                    rho_s = stage.tile([P, OC], F32, name="rho_s", tag="rho_s")
                    eps_s = stage.tile([P, OC], F32, name="eps_s", tag="eps_s")
                    mu_s = stage.tile([P, OC], F32, name="mu_s", tag="mu_s")
                    exp_s = stage.tile([P, OC], F32, name="exp_s", tag="exp_s")
                    sp_s = stage.tile([P, OC], F32, name="sp_s", tag="sp_s")
                    nc.sync.dma_start(rho_s, wrhoT[ksl, sl])
                    nc.sync.dma_start(eps_s, wepsT[ksl, sl])
                    nc.sync.dma_start(mu_s, wmuT[ksl, sl])
                    softplus_fma(
                        w_tiles[k][:, sl], rho_s, eps_s, mu_s, exp_s, sp_s, exp_s
                    )

            # ── main matmul loop
            for m in range(MT):
                msl = bass.ts(m, P)
                xt = xpool.tile([P, KT, P], BF16, name="xt", tag="xt")
                nc.sync.dma_start(xt, xT_r[:, :, msl])
                ps = psump.tile([P, O_PER], F32, name="ps", tag="ps")
                for k in range(KT):
                    for j in range(NOC):
                        jsl = bass.ts(j, OC)
                        nc.tensor.matmul(
                            ps[:, jsl],
                            xt[:, k, :],
                            w_tiles[k][:, jsl],
                            start=(k == 0),
                            stop=(k == KT - 1),
                        )
                out_sb = opool.tile([P, O_PER], F32, name="out_sb", tag="out_sb")
                nc.vector.scalar_tensor_tensor(
                    out_sb, ps, 1.0, bias_bc, op0=ALU.bypass, op1=ALU.add
                )
                nc.sync.dma_start(y[msl, :], out_sb)

    _split_sync_waits(nc)
    nc.finalize()
    return nc


_NC_CACHE = None


def _get_nc():
    global _NC_CACHE
    if _NC_CACHE is None:
        _NC_CACHE = _build()
    return _NC_CACHE


def prepare_in_maps(x, weight_mu, weight_rho, weight_eps, bias_mu, bias_rho, bias_eps):
    x = np.asarray(x, dtype=np.float32)
    weight_mu = np.asarray(weight_mu, dtype=np.float32)
    weight_rho = np.asarray(weight_rho, dtype=np.float32)
    weight_eps = np.asarray(weight_eps, dtype=np.float32)
    bias_mu = np.asarray(bias_mu, dtype=np.float32)
    bias_rho = np.asarray(bias_rho, dtype=np.float32)
    bias_eps = np.asarray(bias_eps, dtype=np.float32)

    xT = np.ascontiguousarray(x.T).astype(ml_dtypes.bfloat16)  # [IN_F, N_TOK]
    in_maps = []
    for c in range(N_CORES):
        osl = slice(c * O_PER, (c + 1) * O_PER)
        in_maps.append(
            {
                "xT": xT,
                "wmuT": np.ascontiguousarray(weight_mu[osl, :].T),
                "wrhoT": np.ascontiguousarray(weight_rho[osl, :].T),
                "wepsT": np.ascontiguousarray(weight_eps[osl, :].T),
                "bmu": bias_mu[osl].reshape(1, O_PER),
                "brho": bias_rho[osl].reshape(1, O_PER),
                "beps": bias_eps[osl].reshape(1, O_PER),
            }
        )
    return in_maps


def run(in_maps, trace=False):
    nc = _get_nc()
    res = run_bass_kernel_spmd(nc, in_maps, list(range(N_CORES)), trace=trace)
    out = np.concatenate([res.results[c]["y"] for c in range(N_CORES)], axis=1)
    return out, res


def kernel(**inputs) -> np.ndarray:
    in_maps = prepare_in_maps(**inputs)
    out, _ = run(in_maps, trace=False)
    return out


# revision 7
# speedup vs baseline: 1.1056x; 1.1056x over previous
"""Bayesian linear layer on 8 Trainium2 NeuronCores.

Computes: weight = mu + softplus(rho) * eps  (elementwise, [O, I])
          bias   = b_mu + softplus(b_rho) * b_eps              ([O])
          y      = x @ weight.T + bias       ([N, I] @ [I, O] -> [N, O])

Shapes: x [8192, 4096], weight_* [16384, 4096], bias_* [16384].

Sharding: column-parallel over 8 cores — each core owns 2048 output
features (its slice of the weight/bias params), x is replicated. Each
core computes an independent [8192, 2048] output slice; the host
concatenates along the feature dim. No collectives needed.

Device kernel (SPMD, one Bass program, per-core data):
 - softplus computed as Ln(Exp(rho) + 1) on the scalar engine (the
   container's act tables lack a direct softplus entry).
 - weights materialized on-chip into resident bf16 SBUF tiles
   [128 i-part, 2048 o] x 32 k-tiles (128 KB/partition).
 - x streamed as transposed bf16 tiles [128 i-part, 32 kt, 128 n];
   host pre-transposes x (both matmul operands need the contraction
   dim on partitions; DMA transpose only supports 2-byte dtypes and
   strided f32 gathers are far off line-rate).
 - matmul: out[n, o] += xT_tile.T @ w_tile, PSUM [128 n, 2048 o]
   (4 banks), 32-step K accumulation, bias added during the PSUM->SBUF
   copy (one DVE pass), then DMA to DRAM.
"""

import numpy as np
import ml_dtypes

import concourse.bass as bass
import concourse.mybir as mybir
import concourse.tile as tile
from concourse.bass_utils import run_bass_kernel_spmd
from concourse.vector_clock import ScopedClock, VectorClock

N_CORES = 8
N_TOK = 8192
IN_F = 4096
OUT_F = 16384
O_PER = OUT_F // N_CORES  # 2048 out features per core

P = 128
KT = IN_F // P       # 32 k-tiles
MT = N_TOK // P      # 64 m-tiles
OC = 512             # o-chunk for weight materialization + matmul N
NOC = O_PER // OC    # 4 o-chunks

F32 = mybir.dt.float32
BF16 = mybir.dt.bfloat16
AF = mybir.ActivationFunctionType
ALU = mybir.AluOpType


def _patch_tile_drain():
    """The walrus build here caps sync-wait commands per CTRL_NO_STRUCT
    instruction; Tile's kernel-tail Drain overflows it. Spread the waits
    across nop carriers (one wait each) before the drain."""
    if getattr(tile.TileContext, "_drain_patched", False):
        return

    def _drain_and_barrier(self, tick_clock, wait_clock):
        nc = self.nc
        gc = tick_clock.global_clock
        n = len(gc)
        for i in range(n):
            t = gc[i]
            if t > 0:
                sub = [0] * n
                sub[i] = t
                carrier = nc.sync.nop(nofuse=True)
                wait_clock.add_sem_waits(
                    carrier.ins, ScopedClock({None: VectorClock(sub)})
                )
        nc.sync.drain()
        nc.all_engine_barrier()
        popped = nc._tile_sem_poison_stack.pop()
        assert popped is self._sem_poison
        nc.clear_and_free_semaphores(list(self.sems.allocated().values()))
        nc.all_engine_barrier()

    tile.TileContext._drain_and_barrier = _drain_and_barrier
    tile.TileContext._drain_patched = True


def _split_sync_waits(nc, max_waits=1):
    """This container's walrus build accepts at most ONE sync-wait command
    per instruction (a 2-wait TensorTensor fails codegen with 'Too many
    sync wait commands'). Tile emits up to 3. Spill the excess onto
    same-engine InstNoOp carriers inserted immediately before the
    overloaded instruction — same-engine program order preserves the
    wait-before-execute semantics."""
    n_spilled = 0
    for fn in nc.m.functions:
        for bb in fn.blocks:
            insts = list(bb.instructions)
            out = []
            changed = False
            for inst in insts:
                si = inst.sync_info
                if si is not None and si.on_wait and len(si.on_wait) > max_waits:
                    waits = list(si.on_wait)
                    spill, keep = waits[:-max_waits], waits[-max_waits:]
                    for w in spill:
                        nop = mybir.InstNoOp(
                            name=f"I-waitspill-{nc.next_id()}", ins=[], outs=[]
                        )
                        nop.engine = inst.engine
                        nop.sync_info = mybir.SyncInfo(on_wait=[w], on_update=[])
                        out.append(nop)
                        n_spilled += 1
                    inst.sync_info = mybir.SyncInfo(
                        on_wait=keep, on_update=list(si.on_update)
                    )
                    changed = True
                out.append(inst)
            if changed:
                bb.instructions = out
    return n_spilled


M_CHUNK = 256            # tokens per x tile (2 lhsT subtiles of 128)
MC = N_TOK // M_CHUNK    # 32 m-chunks
MSUB = M_CHUNK // P      # 2


def _build():
    """o-blocked schedule: for each 512-col output block, stream all 8192
    tokens through the matmul while the NEXT block's params load+softplus
    in parallel. Only the first block's materialization (~25 MB) gates PE
    start; after that the PE streams gap-free (HAM stays warm)."""
    _patch_tile_drain()
    nc = bass.Bass()

    xT = nc.dram_tensor("xT", [IN_F, N_TOK], BF16, kind="ExternalInput")
    wmuT = nc.dram_tensor("wmuT", [IN_F, O_PER], F32, kind="ExternalInput")
    wrhoT = nc.dram_tensor("wrhoT", [IN_F, O_PER], F32, kind="ExternalInput")
    wepsT = nc.dram_tensor("wepsT", [IN_F, O_PER], F32, kind="ExternalInput")
    bmu = nc.dram_tensor("bmu", [1, O_PER], F32, kind="ExternalInput")
    brho = nc.dram_tensor("brho", [1, O_PER], F32, kind="ExternalInput")
    beps = nc.dram_tensor("beps", [1, O_PER], F32, kind="ExternalInput")
    y = nc.dram_tensor("y", [N_TOK, O_PER], F32, kind="ExternalOutput")

    xT_r = xT[:, :].rearrange("(kt p) n -> p kt n", p=P)

    with tile.TileContext(nc) as tc:
        with (
            tc.tile_pool(name="wpool", bufs=1) as wpool,
            tc.tile_pool(name="stage", bufs=2) as stage,
            tc.tile_pool(name="xpool", bufs=2) as xpool,
            tc.tile_pool(name="opool", bufs=4) as opool,
            tc.tile_pool(name="bpool", bufs=1) as bpool,
            tc.tile_pool(name="psum", bufs=4, space="PSUM") as psump,
        ):
            # double-buffered resident weights: parity p holds o-block j
            # with j%2==p. 2 x 32 x [128, 512] bf16 = 64 KB/partition.
            w_tiles = {
                (p, k): wpool.tile([P, OC], BF16, name=f"w_{p}_{k}", tag=f"w_{p}_{k}")
                for p in range(2)
                for k in range(KT)
            }

            bias_bc = bpool.tile([P, O_PER], F32, name="bias_bc")

            def softplus_fma(dst, rho_src, eps_src, mu_src, exp_t, sp_t, prod_t):
                # dst = mu + softplus(rho) * eps, via Ln(Exp(rho) + 1)
                nc.scalar.activation(exp_t, rho_src, AF.Exp)
                nc.scalar.activation(sp_t, exp_t, AF.Ln, bias=1.0)
                nc.vector.tensor_mul(prod_t, sp_t, eps_src)
                nc.vector.tensor_add(dst, prod_t, mu_src)

            def stage_tiles():
                rho_s = stage.tile([P, OC], F32, name="rho_s", tag="rho_s")
                eps_s = stage.tile([P, OC], F32, name="eps_s", tag="eps_s")
                mu_s = stage.tile([P, OC], F32, name="mu_s", tag="mu_s")
                exp_s = stage.tile([P, OC], F32, name="exp_s", tag="exp_s")
                return rho_s, eps_s, mu_s, exp_s

            def materialize_ktile(j, k):
                # w[j%2, k][:, :] = mu + softplus(rho)*eps for o-block j
                jsl = bass.ts(j, OC)
                ksl = slice(k * P, (k + 1) * P)
                rho_s, eps_s, mu_s, exp_s = stage_tiles()
                sp_s = stage.tile([P, OC], F32, name="sp_s", tag="sp_s")
                nc.sync.dma_start(rho_s, wrhoT[ksl, jsl])
                nc.sync.dma_start(eps_s, wepsT[ksl, jsl])
                nc.sync.dma_start(mu_s, wmuT[ksl, jsl])
                softplus_fma(
                    w_tiles[(j % 2, k)], rho_s, eps_s, mu_s, exp_s, sp_s, exp_s
                )

            # ── bias: softplus fma on partition 0, then replicate to all
            # 128 partitions via a doubling SBUF->SBUF DMA ladder (the
            # InstPartitionBroadcast custom op fails codegen here).
            for oc in range(NOC):
                sl = bass.ts(oc, OC)
                rho_s, eps_s, mu_s, exp_s = stage_tiles()
                nc.sync.dma_start(rho_s[0:1, :], brho[0:1, sl])
                nc.sync.dma_start(eps_s[0:1, :], beps[0:1, sl])
                nc.sync.dma_start(mu_s[0:1, :], bmu[0:1, sl])
                softplus_fma(
                    bias_bc[0:1, sl], rho_s[0:1, :], eps_s[0:1, :], mu_s[0:1, :],
                    exp_s[0:1, :], rho_s[0:1, :], exp_s[0:1, :],
                )
            rep = 1
            while rep < P:
                nc.sync.dma_start(bias_bc[rep : 2 * rep, :], bias_bc[0:rep, :])
                rep *= 2

            # ── block 0 weights
            for k in range(KT):
                materialize_ktile(0, k)

            # ── main loop: block j matmuls interleaved (in emission order,
            # so DMA queue FIFOs stay fair) with block j+1 materialization
            for j in range(NOC):
                jsl = bass.ts(j, OC)
                for mc in range(MC):
                    xt = xpool.tile([P, KT, M_CHUNK], BF16, name="xt", tag="xt")
                    nc.sync.dma_start(
                        xt, xT_r[:, :, mc * M_CHUNK : (mc + 1) * M_CHUNK]
                    )
                    ps = psump.tile([P, MSUB * OC], F32, name="ps", tag="ps")
                    for k in range(KT):
                        for s in range(MSUB):
                            nc.tensor.matmul(
                                ps[:, bass.ts(s, OC)],
                                xt[:, k, bass.ts(s, P)],
                                w_tiles[(j % 2, k)],
                                start=(k == 0),
                                stop=(k == KT - 1),
                            )
                    for s in range(MSUB):
                        out_sb = opool.tile([P, OC], F32, name="out_sb", tag="out_sb")
                        nc.vector.scalar_tensor_tensor(
                            out_sb,
                            ps[:, bass.ts(s, OC)],
                            1.0,
                            bias_bc[:, jsl],
                            op0=ALU.bypass,
                            op1=ALU.add,
                        )
                        nc.sync.dma_start(
                            y[mc * M_CHUNK + s * P : mc * M_CHUNK + (s + 1) * P, jsl],
                            out_sb,
                        )
                    if j + 1 < NOC:
                        materialize_ktile(j + 1, mc)

    _split_sync_waits(nc)
    nc.finalize()
    return nc


_NC_CACHE = None


def _get_nc():
    global _NC_CACHE
    if _NC_CACHE is None:
        _NC_CACHE = _build()
    return _NC_CACHE


def prepare_in_maps(x, weight_mu, weight_rho, weight_eps, bias_mu, bias_rho, bias_eps):
    x = np.asarray(x, dtype=np.float32)
    weight_mu = np.asarray(weight_mu, dtype=np.float32)
    weight_rho = np.asarray(weight_rho, dtype=np.float32)
    weight_eps = np.asarray(weight_eps, dtype=np.float32)
    bias_mu = np.asarray(bias_mu, dtype=np.float32)
    bias_rho = np.asarray(bias_rho, dtype=np.float32)
    bias_eps = np.asarray(bias_eps, dtype=np.float32)

    xT = np.ascontiguousarray(x.T).astype(ml_dtypes.bfloat16)  # [IN_F, N_TOK]
    in_maps = []
    for c in range(N_CORES):
        osl = slice(c * O_PER, (c + 1) * O_PER)
        in_maps.append(
            {
                "xT": xT,
                "wmuT": np.ascontiguousarray(weight_mu[osl, :].T),
                "wrhoT": np.ascontiguousarray(weight_rho[osl, :].T),
                "wepsT": np.ascontiguousarray(weight_eps[osl, :].T),
                "bmu": bias_mu[osl].reshape(1, O_PER),
                "brho": bias_rho[osl].reshape(1, O_PER),
                "beps": bias_eps[osl].reshape(1, O_PER),
            }
        )
    return in_maps


def run(in_maps, trace=False):
    nc = _get_nc()
    res = run_bass_kernel_spmd(nc, in_maps, list(range(N_CORES)), trace=trace)
    out = np.concatenate([res.results[c]["y"] for c in range(N_CORES)], axis=1)
    return out, res


def kernel(**inputs) -> np.ndarray:
    in_maps = prepare_in_maps(**inputs)
    out, _ = run(in_maps, trace=False)
    return out


# revision 10
# speedup vs baseline: 1.1803x; 1.0675x over previous
"""Bayesian linear layer on 8 Trainium2 NeuronCores.

Computes: weight = mu + softplus(rho) * eps  (elementwise, [O, I])
          bias   = b_mu + softplus(b_rho) * b_eps              ([O])
          y      = x @ weight.T + bias       ([N, I] @ [I, O] -> [N, O])

Shapes: x [8192, 4096], weight_* [16384, 4096], bias_* [16384].

Sharding: column-parallel over 8 cores — each core owns 2048 output
features (its slice of the weight/bias params), x is replicated. Each
core computes an independent [8192, 2048] output slice; the host
concatenates along the feature dim. No collectives needed.

Device kernel (SPMD, one Bass program, per-core data):
 - softplus computed as Ln(Exp(rho) + 1) on the scalar engine (the
   container's act tables lack a direct softplus entry).
 - weights materialized on-chip into resident bf16 SBUF tiles
   [128 i-part, 2048 o] x 32 k-tiles (128 KB/partition).
 - x streamed as transposed bf16 tiles [128 i-part, 32 kt, 128 n];
   host pre-transposes x (both matmul operands need the contraction
   dim on partitions; DMA transpose only supports 2-byte dtypes and
   strided f32 gathers are far off line-rate).
 - matmul: out[n, o] += xT_tile.T @ w_tile, PSUM [128 n, 2048 o]
   (4 banks), 32-step K accumulation, bias added during the PSUM->SBUF
   copy (one DVE pass), then DMA to DRAM.
"""

import numpy as np
import ml_dtypes

import concourse.bass as bass
import concourse.mybir as mybir
import concourse.tile as tile
from concourse.bass_utils import run_bass_kernel_spmd
from concourse.vector_clock import ScopedClock, VectorClock

N_CORES = 8
N_TOK = 8192
IN_F = 4096
OUT_F = 16384
O_PER = OUT_F // N_CORES  # 2048 out features per core

P = 128
KT = IN_F // P       # 32 k-tiles
MT = N_TOK // P      # 64 m-tiles
OC = 512             # o-chunk for weight materialization + matmul N
NOC = O_PER // OC    # 4 o-chunks

F32 = mybir.dt.float32
BF16 = mybir.dt.bfloat16
AF = mybir.ActivationFunctionType
ALU = mybir.AluOpType


def _patch_tile_drain():
    """The walrus build here caps sync-wait commands per CTRL_NO_STRUCT
    instruction; Tile's kernel-tail Drain overflows it. Spread the waits
    across nop carriers (one wait each) before the drain."""
    if getattr(tile.TileContext, "_drain_patched", False):
        return

    def _drain_and_barrier(self, tick_clock, wait_clock):
        nc = self.nc
        gc = tick_clock.global_clock
        n = len(gc)
        for i in range(n):
            t = gc[i]
            if t > 0:
                sub = [0] * n
                sub[i] = t
                carrier = nc.sync.nop(nofuse=True)
                wait_clock.add_sem_waits(
                    carrier.ins, ScopedClock({None: VectorClock(sub)})
                )
        nc.sync.drain()
        nc.all_engine_barrier()
        popped = nc._tile_sem_poison_stack.pop()
        assert popped is self._sem_poison
        nc.clear_and_free_semaphores(list(self.sems.allocated().values()))
        nc.all_engine_barrier()

    tile.TileContext._drain_and_barrier = _drain_and_barrier
    tile.TileContext._drain_patched = True


def _split_sync_waits(nc, max_waits=1):
    """This container's walrus build accepts at most ONE sync-wait command
    per instruction (a 2-wait TensorTensor fails codegen with 'Too many
    sync wait commands'). Tile emits up to 3. Spill the excess onto
    same-engine InstNoOp carriers inserted immediately before the
    overloaded instruction — same-engine program order preserves the
    wait-before-execute semantics."""
    n_spilled = 0
    for fn in nc.m.functions:
        for bb in fn.blocks:
            insts = list(bb.instructions)
            out = []
            changed = False
            for inst in insts:
                si = inst.sync_info
                if si is not None and si.on_wait and len(si.on_wait) > max_waits:
                    waits = list(si.on_wait)
                    spill, keep = waits[:-max_waits], waits[-max_waits:]
                    for w in spill:
                        nop = mybir.InstNoOp(
                            name=f"I-waitspill-{nc.next_id()}", ins=[], outs=[]
                        )
                        nop.engine = inst.engine
                        nop.sync_info = mybir.SyncInfo(on_wait=[w], on_update=[])
                        out.append(nop)
                        n_spilled += 1
                    inst.sync_info = mybir.SyncInfo(
                        on_wait=keep, on_update=list(si.on_update)
                    )
                    changed = True
                out.append(inst)
            if changed:
                bb.instructions = out
    return n_spilled


M_CHUNK = 512            # tokens per x tile (4 lhsT subtiles of 128)
MC = N_TOK // M_CHUNK    # 16 m-chunks
MSUB = M_CHUNK // P      # 4


def _build():
    """o-blocked schedule: for each 512-col output block, stream all 8192
    tokens through the matmul while the NEXT block's params load+softplus
    in parallel. Only the first block's materialization (~25 MB) gates PE
    start; after that the PE streams gap-free (HAM stays warm)."""
    _patch_tile_drain()
    nc = bass.Bass()

    xT = nc.dram_tensor("xT", [IN_F, N_TOK], BF16, kind="ExternalInput")
    wmuT = nc.dram_tensor("wmuT", [IN_F, O_PER], F32, kind="ExternalInput")
    wrhoT = nc.dram_tensor("wrhoT", [IN_F, O_PER], F32, kind="ExternalInput")
    wepsT = nc.dram_tensor("wepsT", [IN_F, O_PER], F32, kind="ExternalInput")
    bmu = nc.dram_tensor("bmu", [1, O_PER], F32, kind="ExternalInput")
    brho = nc.dram_tensor("brho", [1, O_PER], F32, kind="ExternalInput")
    beps = nc.dram_tensor("beps", [1, O_PER], F32, kind="ExternalInput")
    y = nc.dram_tensor("y", [N_TOK, O_PER], F32, kind="ExternalOutput")

    xT_r = xT[:, :].rearrange("(kt p) n -> p kt n", p=P)

    with tile.TileContext(nc) as tc:
        with (
            tc.tile_pool(name="wpool", bufs=1) as wpool,
            tc.tile_pool(name="stage", bufs=2) as stage,
            tc.tile_pool(name="xpool", bufs=2) as xpool,
            tc.tile_pool(name="opool", bufs=4) as opool,
            tc.tile_pool(name="bpool", bufs=1) as bpool,
            tc.tile_pool(name="psum", bufs=2, space="PSUM") as psump,
        ):
            # double-buffered resident weights: parity p holds o-block j
            # with j%2==p. 2 x 32 x [128, 512] bf16 = 64 KB/partition.
            w_tiles = {
                (p, k): wpool.tile([P, OC], BF16, name=f"w_{p}_{k}", tag=f"w_{p}_{k}")
                for p in range(2)
                for k in range(KT)
            }

            bias_bc = bpool.tile([P, O_PER], F32, name="bias_bc")

            def softplus_fma(dst, rho_src, eps_src, mu_src, exp_t, sp_t, prod_t):
                # dst = mu + softplus(rho) * eps, via Ln(Exp(rho) + 1)
                nc.scalar.activation(exp_t, rho_src, AF.Exp)
                nc.scalar.activation(sp_t, exp_t, AF.Ln, bias=1.0)
                nc.vector.tensor_mul(prod_t, sp_t, eps_src)
                nc.vector.tensor_add(dst, prod_t, mu_src)

            def stage_tiles():
                rho_s = stage.tile([P, OC], F32, name="rho_s", tag="rho_s")
                eps_s = stage.tile([P, OC], F32, name="eps_s", tag="eps_s")
                mu_s = stage.tile([P, OC], F32, name="mu_s", tag="mu_s")
                exp_s = stage.tile([P, OC], F32, name="exp_s", tag="exp_s")
                return rho_s, eps_s, mu_s, exp_s

            def materialize_ktile(j, k):
                # w[j%2, k][:, :] = mu + softplus(rho)*eps for o-block j
                jsl = bass.ts(j, OC)
                ksl = slice(k * P, (k + 1) * P)
                rho_s, eps_s, mu_s, exp_s = stage_tiles()
                sp_s = stage.tile([P, OC], F32, name="sp_s", tag="sp_s")
                nc.sync.dma_start(rho_s, wrhoT[ksl, jsl])
                nc.sync.dma_start(eps_s, wepsT[ksl, jsl])
                nc.sync.dma_start(mu_s, wmuT[ksl, jsl])
                softplus_fma(
                    w_tiles[(j % 2, k)], rho_s, eps_s, mu_s, exp_s, sp_s, exp_s
                )

            # ── bias: softplus fma on partition 0, then replicate to all
            # 128 partitions via a doubling SBUF->SBUF DMA ladder (the
            # InstPartitionBroadcast custom op fails codegen here).
            for oc in range(NOC):
                sl = bass.ts(oc, OC)
                rho_s, eps_s, mu_s, exp_s = stage_tiles()
                nc.sync.dma_start(rho_s[0:1, :], brho[0:1, sl])
                nc.sync.dma_start(eps_s[0:1, :], beps[0:1, sl])
                nc.sync.dma_start(mu_s[0:1, :], bmu[0:1, sl])
                softplus_fma(
                    bias_bc[0:1, sl], rho_s[0:1, :], eps_s[0:1, :], mu_s[0:1, :],
                    exp_s[0:1, :], rho_s[0:1, :], exp_s[0:1, :],
                )
            rep = 1
            while rep < P:
                nc.sync.dma_start(bias_bc[rep : 2 * rep, :], bias_bc[0:rep, :])
                rep *= 2

            # ── block 0 weights
            for k in range(KT):
                materialize_ktile(0, k)

            # ── main loop: block j matmuls interleaved (in emission order,
            # so DMA queue FIFOs stay fair) with block j+1 materialization
            for j in range(NOC):
                jsl = bass.ts(j, OC)
                for mc in range(MC):
                    xt = xpool.tile([P, KT, M_CHUNK], BF16, name="xt", tag="xt")
                    nc.sync.dma_start(
                        xt, xT_r[:, :, mc * M_CHUNK : (mc + 1) * M_CHUNK]
                    )
                    ps = psump.tile([P, MSUB * OC], F32, name="ps", tag="ps")
                    for k in range(KT):
                        for s in range(MSUB):
                            nc.tensor.matmul(
                                ps[:, bass.ts(s, OC)],
                                xt[:, k, bass.ts(s, P)],
                                w_tiles[(j % 2, k)],
                                start=(k == 0),
                                stop=(k == KT - 1),
                            )
                    for s in range(MSUB):
                        out_sb = opool.tile([P, OC], F32, name="out_sb", tag="out_sb")
                        nc.vector.scalar_tensor_tensor(
                            out_sb,
                            ps[:, bass.ts(s, OC)],
                            1.0,
                            bias_bc[:, jsl],
                            op0=ALU.bypass,
                            op1=ALU.add,
                        )
                        nc.sync.dma_start(
                            y[mc * M_CHUNK + s * P : mc * M_CHUNK + (s + 1) * P, jsl],
                            out_sb,
                        )
                    if j + 1 < NOC:
                        for dk in range(KT // MC):
                            materialize_ktile(j + 1, mc * (KT // MC) + dk)

    _split_sync_waits(nc)
    nc.finalize()
    return nc


_NC_CACHE = None


def _get_nc():
    global _NC_CACHE
    if _NC_CACHE is None:
        _NC_CACHE = _build()
    return _NC_CACHE


def prepare_in_maps(x, weight_mu, weight_rho, weight_eps, bias_mu, bias_rho, bias_eps):
    x = np.asarray(x, dtype=np.float32)
    weight_mu = np.asarray(weight_mu, dtype=np.float32)
    weight_rho = np.asarray(weight_rho, dtype=np.float32)
    weight_eps = np.asarray(weight_eps, dtype=np.float32)
    bias_mu = np.asarray(bias_mu, dtype=np.float32)
    bias_rho = np.asarray(bias_rho, dtype=np.float32)
    bias_eps = np.asarray(bias_eps, dtype=np.float32)

    xT = np.ascontiguousarray(x.T).astype(ml_dtypes.bfloat16)  # [IN_F, N_TOK]
    in_maps = []
    for c in range(N_CORES):
        osl = slice(c * O_PER, (c + 1) * O_PER)
        in_maps.append(
            {
                "xT": xT,
                "wmuT": np.ascontiguousarray(weight_mu[osl, :].T),
                "wrhoT": np.ascontiguousarray(weight_rho[osl, :].T),
                "wepsT": np.ascontiguousarray(weight_eps[osl, :].T),
                "bmu": bias_mu[osl].reshape(1, O_PER),
                "brho": bias_rho[osl].reshape(1, O_PER),
                "beps": bias_eps[osl].reshape(1, O_PER),
            }
        )
    return in_maps


def run(in_maps, trace=False):
    nc = _get_nc()
    res = run_bass_kernel_spmd(nc, in_maps, list(range(N_CORES)), trace=trace)
    out = np.concatenate([res.results[c]["y"] for c in range(N_CORES)], axis=1)
    return out, res


def kernel(**inputs) -> np.ndarray:
    in_maps = prepare_in_maps(**inputs)
    out, _ = run(in_maps, trace=False)
    return out
